# revision 1
# baseline (speedup 1.0000x reference)
"""MoE BasicBlock kernel for TRN2: W = sum_e alpha_e * w_e (21 experts), then
conv3x3 -> BN -> relu -> conv3x3 -> BN -> +x -> relu on x [N,256,56,56] f32.

Data-parallel across 8 NeuronCores: 4 images per core. Convs run as 9-position
bf16 matmul accumulation in PSUM (contraction over input-channel chunks of
128); the alpha-combine of expert weights is done on-device (redundantly per
core), followed by a PE transpose into lhsT layout.
"""

import numpy as np

import concourse.bass as bass
import concourse.mybir as mybir
import concourse.tile as tile
from concourse.masks import make_identity

FP32 = mybir.dt.float32
BF16 = mybir.dt.bfloat16

C = 256  # channels
CCH = 2  # channel chunks of 128
H = W = 56
HP = WP = 58  # padded
E = 21  # experts
ROWT = 8  # output rows per psum tile
NRT = H // ROWT  # 7 row tiles
NTILE = ROWT * W  # 448
KHW = 9  # kernel positions
IC9 = C * KHW  # 2304, per-oc-row weight elements
EPS = 1e-5


def split_multi_waits(nc):
    """The installed walrus accepts at most one sync-wait per instruction
    (two for EventSemaphore). Tile's sem assignment can emit more; split the
    extras onto injected same-engine nops placed immediately before the
    offending instruction (equivalent semantics for in-order engine streams).
    """
    n_split = 0
    n_dma_split = 0
    for bb in nc.main_func.blocks:
        new_list = []
        for inst in list(bb.instructions):
            si = inst.sync_info
            waits = list(si.on_wait) if si is not None and si.on_wait else []
            cap = 2 if isinstance(inst, mybir.InstEventSemaphore) else 1
            if len(waits) > cap:
                if getattr(inst, "queue", None) is not None:
                    n_dma_split += 1
                extra, keep = waits[:-cap], waits[-cap:]
                for w in extra:
                    nop = nc.engines[inst.engine].nop(hint="waitsplit", nofuse=True)
                    # nop() appended itself to nc.cur_bb; pull it back out.
                    host_bb = nc.cur_bb
                    assert host_bb.bb.instructions[-1] is nop.ins
                    host_bb.bb.instructions.pop()
                    nop.ins.sync_info = mybir.SyncInfo(on_update=[], on_wait=[w])
                    new_list.append(nop.ins)
                    n_split += 1
                inst.sync_info = mybir.SyncInfo(
                    on_update=list(si.on_update) if si.on_update else [], on_wait=keep
                )
            new_list.append(inst)
        bb.instructions[:] = new_list
    return n_split, n_dma_split


def build_nc(npc=4, repeat=1, sharded=True, n_cores=8):
    """Build the per-core Bass program. npc = images per core. repeat>1
    re-emits the whole computation (for differential wall-clock timing).
    sharded=True: each core alpha-combines only C/n_cores output channels of
    each conv weight (inputs w1s/w2s are per-core slices) and the full
    combined weights are assembled with an 8-core AllGather."""
    nc = bass.Bass(
        "TRN2", target_bir_lowering=False, debug=False,
        num_devices=n_cores if sharded else 1,
    )

    x = nc.dram_tensor("x", [npc, C, H, W], FP32, kind="ExternalInput")
    alpha = nc.dram_tensor("alpha", [E], FP32, kind="ExternalInput")
    OSH = C // n_cores  # oc rows combined per core (32)
    if sharded:
        w1 = nc.dram_tensor("w1s", [E, OSH, C, 3, 3], FP32, kind="ExternalInput")
        w2 = nc.dram_tensor("w2s", [E, OSH, C, 3, 3], FP32, kind="ExternalInput")
    else:
        w1 = nc.dram_tensor("w1", [E, C, C, 3, 3], FP32, kind="ExternalInput")
        w2 = nc.dram_tensor("w2", [E, C, C, 3, 3], FP32, kind="ExternalInput")
    bn = {}
    for nm in ("g1", "b1", "m1", "v1", "g2", "b2", "m2", "v2"):
        bn[nm] = nc.dram_tensor(nm, [C], FP32, kind="ExternalInput")
    out = nc.dram_tensor("out", [npc, C, H, W], FP32, kind="ExternalOutput")

    xap = x.ap().rearrange("n c h w -> n c (h w)")
    oap = out.ap().rearrange("n c h w -> n c (h w)")
    w1ap = w1.ap().rearrange("e o i h w -> e o (i h w)")
    w2ap = w2.ap().rearrange("e o i h w -> e o (i h w)")
    if sharded:
        wparts = [nc.dram_tensor(f"wpart{i}", [OSH * IC9], FP32) for i in range(2)]
        wgaths = [
            nc.dram_tensor(f"wgath{i}", [C, IC9], FP32, addr_space="Shared")
            for i in range(2)
        ]

    with tile.TileContext(nc) as tc:
        import contextlib

        with contextlib.ExitStack() as ctx:
            singles = ctx.enter_context(tc.tile_pool(name="singles", bufs=1))
            epool = ctx.enter_context(tc.tile_pool(name="epool", bufs=3))
            wfpool = ctx.enter_context(tc.tile_pool(name="wfpool", bufs=2))
            xspool = ctx.enter_context(tc.tile_pool(name="xspool", bufs=2))
            xpads = ctx.enter_context(tc.tile_pool(name="xpads", bufs=4))
            ypads = ctx.enter_context(tc.tile_pool(name="ypads", bufs=6))
            xrpool = ctx.enter_context(tc.tile_pool(name="xrpool", bufs=6))
            obpool = ctx.enter_context(tc.tile_pool(name="obpool", bufs=4))
            cpsum = ctx.enter_context(tc.tile_pool(name="cpsum", bufs=6, space="PSUM"))
            tpsum = ctx.enter_context(tc.tile_pool(name="tpsum", bufs=2, space="PSUM"))

            # ---- stage 0: BN params, alpha, identity ----
            ident = singles.tile([128, 128], FP32, tag="ident")
            make_identity(nc, ident[:])

            zero_c = singles.tile([128, 1], FP32, tag="zero_c")
            nc.vector.memset(zero_c[:], 0.0)
            nc.const_aps.aps[(FP32, 0.0)] = zero_c[:]
            eps_c = singles.tile([128, 1], FP32, tag="eps_c")
            nc.vector.memset(eps_c[:], EPS)

            alpha_sb = singles.tile([128, E], FP32, tag="alpha")
            nc.sync.dma_start(
                out=alpha_sb[:],
                in_=bass.AP(tensor=alpha.ap().tensor, offset=0, ap=[[0, 128], [1, E]]),
            )

            bns = {}
            for nm in ("g1", "b1", "m1", "v1", "g2", "b2", "m2", "v2"):
                t = singles.tile([128, CCH], FP32, name=f"bn_{nm}", tag=f"bn_{nm}")
                nc.sync.dma_start(
                    out=t[:],
                    in_=bass.AP(
                        tensor=bn[nm].ap().tensor, offset=0, ap=[[1, 128], [128, CCH]]
                    ),
                )
                bns[nm] = t

            def bn_fold(g, b, m, v, idx):
                # s = g / sqrt(v + eps) (one Newton step on ACT sqrt),
                # b' = b - m * s
                sq = singles.tile([128, CCH], FP32, name=f"bn_sq{idx}", tag=f"bn_sq{idx}")
                nc.scalar.activation(
                    sq[:], v[:], mybir.ActivationFunctionType.Sqrt, bias=eps_c[:]
                )
                r = singles.tile([128, CCH], FP32, name=f"bn_r{idx}", tag=f"bn_r{idx}")
                nc.vector.reciprocal(r[:], sq[:])
                # Newton: y1 = 0.5*(y0 + x/y0); x/y0 = (v+eps)*r
                ve = singles.tile([128, CCH], FP32, name=f"bn_ve{idx}", tag=f"bn_ve{idx}")
                nc.vector.tensor_scalar_add(ve[:], v[:], EPS)
                t1 = singles.tile([128, CCH], FP32, name=f"bn_t1{idx}", tag=f"bn_t1{idx}")
                nc.vector.tensor_mul(t1[:], ve[:], r[:])
                nc.vector.tensor_add(t1[:], t1[:], sq[:])
                nc.vector.tensor_scalar_mul(t1[:], t1[:], 0.5)  # refined sqrt
                nc.vector.reciprocal(r[:], t1[:])  # refined rsqrt
                s = singles.tile([128, CCH], FP32, name=f"bn_s{idx}", tag=f"bn_s{idx}")
                nc.vector.tensor_mul(s[:], g[:], r[:])
                bp = singles.tile([128, CCH], FP32, name=f"bn_bp{idx}", tag=f"bn_bp{idx}")
                nc.vector.tensor_mul(bp[:], m[:], s[:])
                nc.vector.tensor_sub(bp[:], b[:], bp[:])
                return s, bp

            s1, b1p = bn_fold(bns["g1"], bns["b1"], bns["m1"], bns["v1"], 1)
            s2, b2p = bn_fold(bns["g2"], bns["b2"], bns["m2"], bns["v2"], 2)

            # ---- weight combine + transpose to lhsT ----
            # lhsT[wi][ic][oc]: [128(ic), 9, 128(oc)] bf16
            lhsT = [
                [
                    [
                        singles.tile([128, KHW, 128], BF16, name=f"lhsT_{wi}_{ic}_{oc}", tag=f"lhsT_{wi}_{ic}_{oc}")
                        for oc in range(CCH)
                    ]
                    for ic in range(CCH)
                ]
                for wi in range(2)
            ]

            def combine_to_wf(wap, prescale, oc):
                wf = wfpool.tile([128, IC9], FP32, name="wfull", tag="wfull")
                for e in range(E):
                    est = epool.tile([128, IC9], FP32, name="estage", tag="estage")
                    nc.sync.dma_start(
                        out=est[:], in_=wap[e, oc * 128 : (oc + 1) * 128, :]
                    )
                    if e == 0:
                        nc.vector.tensor_scalar_mul(wf[:], est[:], alpha_sb[:, 0:1])
                    else:
                        nc.vector.scalar_tensor_tensor(
                            wf[:],
                            est[:],
                            alpha_sb[:, e : e + 1],
                            wf[:],
                            op0=mybir.AluOpType.mult,
                            op1=mybir.AluOpType.add,
                        )
                if prescale is not None:
                    nc.vector.tensor_scalar_mul(wf[:], wf[:], prescale[:, oc : oc + 1])
                return wf

            def transpose_wf(wi, oc, wf):
                wfr = wf[:].rearrange("p (c i r) -> p c i r", c=CCH, r=KHW)
                for ic in range(CCH):
                    for pos in range(KHW):
                        pt = tpsum.tile([128, 128], FP32, name="tpsum", tag="tpsum")
                        nc.tensor.transpose(pt[:], wfr[:, ic, :, pos], ident[:])
                        nc.vector.tensor_copy(lhsT[wi][ic][oc][:, pos, :], pt[:])

            def combine_weights(wi, wap, prescale):
                for oc in range(CCH):
                    transpose_wf(wi, oc, combine_to_wf(wap, prescale, oc))

            SFREE = (C // n_cores) * IC9 // 128  # 576

            def shard_combine_gather(wi, wap):
                # per-core slice [E, 32, IC9] -> flat [E, 128, 576]
                acc = wfpool.tile([128, SFREE], FP32, name=f"sacc{wi}", tag="sacc")
                for e in range(E):
                    est = epool.tile([128, SFREE], FP32, name="sest", tag="sest")
                    flat = wap[e].rearrange("o f -> (o f)").rearrange(
                        "(p f) -> p f", p=128
                    )
                    nc.sync.dma_start(out=est[:], in_=flat)
                    if e == 0:
                        nc.vector.tensor_scalar_mul(acc[:], est[:], alpha_sb[:, 0:1])
                    else:
                        nc.vector.scalar_tensor_tensor(
                            acc[:], est[:], alpha_sb[:, e : e + 1], acc[:],
                            op0=mybir.AluOpType.mult, op1=mybir.AluOpType.add,
                        )
                wp = wparts[wi].ap().rearrange("(p f) -> p f", p=128)
                nc.sync.dma_start(out=wp, in_=acc[:])
                nc.gpsimd.collective_compute(
                    "AllGather",
                    mybir.AluOpType.bypass,
                    replica_groups=[list(range(n_cores))],
                    ins=[wparts[wi].ap().opt()],
                    outs=[wgaths[wi].ap().rearrange("a b -> (a b)").opt()],
                )

            def load_transpose_gathered(wi, prescale):
                for oc in range(CCH):
                    wf = wfpool.tile([128, IC9], FP32, name="wfull", tag="wfull")
                    nc.sync.dma_start(
                        out=wf[:], in_=wgaths[wi].ap()[oc * 128 : (oc + 1) * 128, :]
                    )
                    if prescale is not None:
                        nc.vector.tensor_scalar_mul(
                            wf[:], wf[:], prescale[:, oc : oc + 1]
                        )
                    transpose_wf(wi, oc, wf)

            # ---- xpad fill ----
            def fill_xpad(n):
                tiles = []
                for c in range(CCH):
                    xst = xspool.tile([128, H * W], FP32, name="xstage", tag="xstage")
                    nc.sync.dma_start(
                        out=xst[:], in_=xap[n, c * 128 : (c + 1) * 128, :]
                    )
                    xp = xpads.tile([128, HP * WP], BF16, name="xpad", tag="xpad")
                    xpr = xp[:].rearrange("p (r c) -> p r c", r=HP)
                    zero_ring(xpr)
                    nc.scalar.copy(
                        out=xpr[:, 1 : H + 1, 1 : W + 1],
                        in_=xst[:].rearrange("p (r c) -> p r c", r=H),
                    )
                    tiles.append(xp)
                return tiles

            def zero_ring(tr):
                # tr: [128, HP, WP] view (DVE: gpsimd stream hosts collectives)
                nc.vector.memset(tr[:, 0, :], 0.0)
                nc.vector.memset(tr[:, HP - 1, :], 0.0)
                nc.vector.memset(tr[:, 1 : HP - 1, 0:1], 0.0)
                nc.vector.memset(tr[:, 1 : HP - 1, WP - 1 : WP], 0.0)

            def conv(n, src_tiles, wi, dst):
                """One conv3x3 over image n. src_tiles: [128,HP*WP] bf16 per ic
                chunk. dst: ypads tiles (wi=0) or DRAM out via epilogue (wi=1)."""
                srcv = [
                    t[:].rearrange("p (r c) -> p r c", r=HP) for t in src_tiles
                ]
                for oc in range(CCH):
                    for rt in range(NRT):
                        ps = cpsum.tile([128, NTILE], FP32, name="cpsum", tag="cpsum")
                        k = 0
                        for ic in range(CCH):
                            for ky in range(3):
                                for kx in range(3):
                                    rhs = srcv[ic][
                                        :, rt * ROWT + ky : rt * ROWT + ky + ROWT,
                                        kx : kx + W,
                                    ]
                                    nc.tensor.matmul(
                                        ps[:],
                                        lhsT[wi][ic][oc][:, 3 * ky + kx, :],
                                        rhs,
                                        start=(k == 0),
                                        stop=(k == 17),
                                    )
                                    k += 1
                        psr = ps[:].rearrange("p (r c) -> p r c", r=ROWT)
                        if wi == 0:
                            ypr = dst[oc][:].rearrange("p (r c) -> p r c", r=HP)
                            nc.scalar.activation(
                                ypr[:, rt * ROWT + 1 : rt * ROWT + 1 + ROWT, 1 : W + 1],
                                psr,
                                mybir.ActivationFunctionType.Relu,
                                bias=b1p[:, oc : oc + 1],
                                scale=s1[:, oc : oc + 1],
                            )
                        else:
                            xr = xrpool.tile([128, NTILE], FP32, name="xres", tag="xres")
                            nc.sync.dma_start(
                                out=xr[:],
                                in_=xap[
                                    n,
                                    oc * 128 : (oc + 1) * 128,
                                    rt * NTILE : (rt + 1) * NTILE,
                                ],
                            )
                            ob = obpool.tile([128, NTILE], FP32, name="ob", tag="ob")
                            # ob = (psum + b2') + x
                            nc.vector.scalar_tensor_tensor(
                                ob[:],
                                ps[:],
                                b2p[:, oc : oc + 1],
                                xr[:],
                                op0=mybir.AluOpType.add,
                                op1=mybir.AluOpType.add,
                            )
                            nc.scalar.activation(
                                ob[:], ob[:], mybir.ActivationFunctionType.Relu,
                                bias=zero_c[:],
                            )
                            nc.sync.dma_start(
                                out=oap[
                                    n,
                                    oc * 128 : (oc + 1) * 128,
                                    rt * NTILE : (rt + 1) * NTILE,
                                ],
                                in_=ob[:],
                            )

            def alloc_ypad():
                tiles = []
                for c in range(CCH):
                    yp = ypads.tile([128, HP * WP], BF16, name="ypad", tag="ypad")
                    zero_ring(yp[:].rearrange("p (r c) -> p r c", r=HP))
                    tiles.append(yp)
                return tiles

            # ---- emission schedule (engine streams are in-order) ----
            # PE order: W1T, c1(0..2), W2T, c2(0), c1(3), c2(1..3).
            # W2's expert DMAs are emitted right after W1's so the 100MB
            # expert stream runs back-to-back while PE does conv1; W2's PE
            # transposes are deferred until the combine has surely landed.
            for _rep in range(repeat):
              if sharded:
                  shard_combine_gather(0, w1ap)
                  shard_combine_gather(1, w2ap)
                  load_transpose_gathered(0, None)
              else:
                  combine_weights(0, w1ap, None)
              if npc == 1:
                  xp0 = fill_xpad(0)
                  yp0 = alloc_ypad()
                  conv(0, xp0, 0, yp0)
                  if sharded:
                      load_transpose_gathered(1, s2)
                  else:
                      combine_weights(1, w2ap, s2)
                  conv(0, yp0, 1, None)
              else:
                  assert npc == 4
                  if not sharded:
                      wf2 = [combine_to_wf(w2ap, s2, oc) for oc in range(CCH)]
                  xps = {0: fill_xpad(0), 1: fill_xpad(1)}
                  yps = {}
                  for n in range(3):
                      yps[n] = alloc_ypad()
                      conv(n, xps[n], 0, yps[n])
                      # refills reuse slots released by the conv above
                      if n + 2 < npc:
                          xps[n + 2] = fill_xpad(n + 2)
                      if sharded and n == 0:
                          load_transpose_gathered(1, s2)
                  if not sharded:
                      for oc in range(CCH):
                          transpose_wf(1, oc, wf2[oc])
                  conv(0, yps[0], 1, None)
                  yps[3] = alloc_ypad()
                  conv(3, xps[3], 0, yps[3])
                  for n in range(1, npc):
                      conv(n, yps[n], 1, None)

    n_split, n_dma_split = split_multi_waits(nc)
    return nc, (n_split, n_dma_split)


# ---------------------------------------------------------------------------
# Host-side entry point: takes FULL inputs, shards batch across 8 cores.
# ---------------------------------------------------------------------------
_NC_CACHE = {}


def kernel(**inputs):
    from concourse.bass_utils import run_bass_kernel_spmd

    x = np.ascontiguousarray(np.asarray(inputs["x"], dtype=np.float32))
    n_total = x.shape[0]
    n_cores = 8
    npc = n_total // n_cores
    assert npc * n_cores == n_total

    key = npc
    if key not in _NC_CACHE:
        _NC_CACHE[key] = build_nc(npc=npc)[0]
    nc = _NC_CACHE[key]

    w1 = np.asarray(inputs["w1"], dtype=np.float32)
    w2 = np.asarray(inputs["w2"], dtype=np.float32)
    osh = w1.shape[1] // n_cores
    shared = {
        k: np.ascontiguousarray(np.asarray(v, dtype=np.float32))
        for k, v in inputs.items()
        if k not in ("x", "w1", "w2")
    }
    in_maps = [
        {
            "x": x[c * npc : (c + 1) * npc],
            "w1s": np.ascontiguousarray(w1[:, c * osh : (c + 1) * osh]),
            "w2s": np.ascontiguousarray(w2[:, c * osh : (c + 1) * osh]),
            **shared,
        }
        for c in range(n_cores)
    ]
    res = run_bass_kernel_spmd(nc, in_maps, core_ids=list(range(n_cores)))
    return np.concatenate([res.results[c]["out"] for c in range(n_cores)], axis=0)

